# revision 47
# baseline (speedup 1.0000x reference)
"""Cross-attention kernel for Trainium2, 8 NeuronCores, data-parallel over batch.

Reference computes, per batch b:
    q_proj = q[b] @ Wq; k_proj = y[b] @ Wk; v_proj = k_proj @ Wv
    out    = softmax(q_proj @ k_proj.T / 32) @ v_proj

Weight-folding (host, weight-only algebra):
    M = Wq @ Wk.T        [d_q, d_y]
    N = Wk @ Wv          [d_y, f]
so the device per core computes
    A  = q @ M           [Nq, 1024]      (4.3 GF)
    V  = y @ N           [Nk, 1024]      (4.3 GF)
    ST = yT' A           [m, n] blocks   (8.6 GF)   contract over y's raw d
    P  = exp(ST / 32)    fp16
    O  = (P.T @ V) / (P.T @ 1)           (8.6 GF)
i.e. k_proj never exists on device: 25.8 GF/core instead of 30.1 GF.

Everything runs in fp16 on the PE (same rate as bf16, 4x the mantissa;
sim rel-err 4.5e-4): host ships qT/yT/M/N as fp16 (12 MB of loads vs 28),
output is stored fp16 and upcast on host.

Softmax denominator: DVE accumulates colsum(eT) over the 16 m-chunks in
fp32 (hidden under the S matmuls), then one 1-column matmul per 128-query
block turns it into partition-major layout -- 16 tiny matmuls per core
instead of 256.

All inputs are host-swizzled to partition-major dram layout so each
tensor loads as one DMA with 8-32 KB contiguous per partition (the DMA
engines are packet-rate bound, so packet size sets bandwidth).  M
arrives in four e-block DMAs so the first A matmul needs only 1.5 MB
in flight; 10 zero matmuls warm the PE HAM clock gate while that DMA
runs.  Output normalization is split across VectorE and ScalarE.
"""

import numpy as np
from contextlib import ExitStack

import concourse.bass as bass
import concourse.tile as tile
from concourse import bacc, mybir
from concourse.bass_utils import run_bass_kernel_spmd

P = 128
F32 = mybir.dt.float32
F16 = mybir.dt.float16

B = 8
NQ = 2048
NK = 2048
D = 1024   # in_q_dim == in_dim
E = 1024   # folded inner dim (== y's raw feature dim)
F = 1024   # out_dim

NBLK = 512


def build_program(nq=NQ, nk=NK, d=D, e=E, f=F, nblk=NBLK, warmup_mms=20):
    nc = bacc.Bacc(trn_type="TRN2")

    DC = d // P          # 8   contraction chunks for A/V
    EC = e // P          # 8   contraction chunks for S
    MC = nk // P         # 16  key chunks
    NB = nq // nblk      # 4   query blocks
    NSUB = nblk // P     # 4   128-query sub-blocks
    FJ = f // 512        # 2   value free-dim chunks

    # All inputs are host-swizzled to partition-major layout so each
    # tensor loads as ONE DMA with 8-32 KB contiguous per partition
    # (DMA engines are packet-rate bound: ~80 ns/packet regardless of
    # size, so big packets are everything).
    # block 0 of qT ships separately, h-major in two 256-query halves,
    # so the very first A matmuls need only 1 MB of DMA in flight
    qT0 = nc.dram_tensor("qT0", [P, 2 * DC * 256], F16,
                         kind="ExternalInput").ap()
    qTr = nc.dram_tensor("qTr", [P, (NB - 1) * DC * nblk], F16,
                         kind="ExternalInput").ap()
    yT = nc.dram_tensor("yT", [P, DC * nk], F16, kind="ExternalInput").ap()
    Mf = nc.dram_tensor("M", [P, DC * e], F16, kind="ExternalInput").ap()
    Nf = nc.dram_tensor("N", [P, DC * f], F16, kind="ExternalInput").ap()
    out = nc.dram_tensor("out", [nq, f], F16, kind="ExternalOutput").ap()

    GE = 4                      # M arrives in GE e-block DMAs of e//GE cols
    EB = e // GE                # 256
    qT0_v = qT0.rearrange("p (h c n) -> p h c n", h=2, c=DC)  # [P,2,DC,256]
    qTr_v = qTr.rearrange("p (b c n) -> p b c n", b=NB - 1, c=DC)
    MB = nk // nblk             # yT arrives in MB m-block DMAs
    yT_v = yT.rearrange("p (b c m) -> p b c m", b=MB, c=DC)  # [P,MB,DC,nblk]
    M_v = Mf.rearrange("p (g c e) -> p g c e", g=GE, c=DC)   # [P, GE, DC, EB]
    N_v = Nf.rearrange("p (c f) -> p c f", c=DC)             # [P, DC, f]
    out_v = out.rearrange("(b p) f -> b p f", p=P)           # [nq//P, P, f]

    with tile.TileContext(nc) as tc, ExitStack() as ctx:
        consts = ctx.enter_context(tc.tile_pool(name="consts", bufs=1))
        qt0_pool = ctx.enter_context(tc.tile_pool(name="qt0", bufs=1))
        qt_pool = ctx.enter_context(tc.tile_pool(name="qt", bufs=NB - 1))
        yt_pool = ctx.enter_context(tc.tile_pool(name="yt", bufs=1))
        mn_pool = ctx.enter_context(tc.tile_pool(name="mn", bufs=1))
        v_pool = ctx.enter_context(tc.tile_pool(name="vproj", bufs=1))
        at_pool = ctx.enter_context(tc.tile_pool(name="at", bufs=2))
        et_pool = ctx.enter_context(tc.tile_pool(name="et", bufs=2))
        es_pool = ctx.enter_context(tc.tile_pool(name="esum", bufs=2))
        out_pool = ctx.enter_context(tc.tile_pool(name="outsb", bufs=2))
        small = ctx.enter_context(tc.tile_pool(name="small", bufs=4))
        psum_a = ctx.enter_context(
            tc.tile_pool(name="psum_a", bufs=3, space="PSUM"))
        psum_o = ctx.enter_context(
            tc.tile_pool(name="psum_o", bufs=4, space="PSUM"))
        psum_s = ctx.enter_context(
            tc.tile_pool(name="psum_s", bufs=1, space="PSUM"))

        ones16 = consts.tile([P, 1], F16)
        nc.vector.memset(ones16, 1.0)
        zbias = consts.tile([P, 1], F32)
        nc.vector.memset(zbias, 0.0)
        wscr = consts.tile([P, 256], F16)
        nc.vector.memset(wscr, 0.0)

        M_sb = mn_pool.tile([P, GE, DC, EB], F16)
        N_sb = mn_pool.tile([P, DC, f], F16)
        yT_sb = yt_pool.tile([P, MB, DC, nblk], F16)
        V_sb = v_pool.tile([P, MC, f], F16)
        qt0_sb = qt0_pool.tile([P, 2, DC, 256], F16, tag="qt0")
        qts = [qt_pool.tile([P, DC, nblk], F16, tag="qt", name=f"qt{i}")
               for i in range(1, NB)]

        # ---- DMA schedule: one queue, whole-tensor transfers
        # (partition-major dram layout -> max packet size), ordered by
        # first consumption: A0 needs M-block0+qt0, S0 needs yT, V
        # needs N.  M comes in GE e-blocks so A0 starts after 1.5 MB.
        nc.sync.dma_start(M_sb[:, 0], M_v[:, 0])
        nc.sync.dma_start(qt0_sb[:, 0], qT0_v[:, 0])
        nc.sync.dma_start(M_sb[:, 1], M_v[:, 1])
        nc.sync.dma_start(qt0_sb[:, 1], qT0_v[:, 1])
        for g in range(2, GE):
            nc.sync.dma_start(M_sb[:, g], M_v[:, g])
        nc.sync.dma_start(N_sb, N_v)
        for mb in range(MB):
            nc.sync.dma_start(yT_sb[:, mb], yT_v[:, mb])
        for nb in range(1, NB):
            nc.sync.dma_start(qts[nb - 1], qTr_v[:, nb - 1])

        # ---- warm the PE clock gate while DMA streams in ----
        if warmup_mms:
            wps = psum_s.tile([P, 256], F32, tag="pss", name="warm")
            for _ in range(warmup_mms):
                nc.tensor.matmul(wps, lhsT=wscr[:, 0:P], rhs=wscr,
                                 start=True, stop=True)

        epg = EB // P   # e'-slices per M e-block

        def a0_phase(at):
            # block 0: two 256-query halves so compute starts on 1 MB
            for h in range(2):
                for ei in range(EC):
                    ps = psum_a.tile([P, 512], F32, tag="psa",
                                     name="psa")[:, :256]
                    g, eo = ei // epg, (ei % epg) * P
                    for di in range(DC):
                        nc.tensor.matmul(
                            ps, lhsT=M_sb[:, g, di, eo:eo + P],
                            rhs=qt0_sb[:, h, di, :],
                            start=(di == 0), stop=(di == DC - 1))
                    nc.vector.tensor_copy(
                        at[:, ei, h * 256:(h + 1) * 256], ps)

        def a_phase(nb, at):
            # AT[e', nblk] = M.T @ qT_block   (psum partition = e'-slice)
            qt = qts[nb - 1]
            for ei in range(EC):
                ps = psum_a.tile([P, 512], F32, tag="psa", name="psa")
                g, eo = ei // epg, (ei % epg) * P
                for di in range(DC):
                    nc.tensor.matmul(
                        ps, lhsT=M_sb[:, g, di, eo:eo + P],
                        rhs=qt[:, di, :],
                        start=(di == 0), stop=(di == DC - 1))
                nc.vector.tensor_copy(at[:, ei, :], ps)

        def yt_sl(mi, ci):
            # m-slice mi of chunk ci in the m-blocked yT layout
            mb, r = mi // (nblk // P), (mi % (nblk // P)) * P
            return yT_sb[:, mb, ci, r:r + P]

        def v_phase():
            # V[m, f] = yT.T @ N   (psum partition = m-slice)
            for mi in range(MC):
                for fj in range(FJ):
                    ps = psum_a.tile([P, 512], F32, tag="psa", name="psa")
                    for di in range(DC):
                        nc.tensor.matmul(
                            ps, lhsT=yt_sl(mi, di),
                            rhs=N_sb[:, di, fj * 512:(fj + 1) * 512],
                            start=(di == 0), stop=(di == DC - 1))
                    nc.vector.tensor_copy(
                        V_sb[:, mi, fj * 512:(fj + 1) * 512], ps)

        def s_phase(at, et, esum):
            # eT[m, nblk] = exp((yT.T A) / 32); esum = colsum over m-chunks
            for mi in range(MC):
                ps = psum_a.tile([P, 512], F32, tag="psa", name="psa")
                for ei in range(EC):
                    nc.tensor.matmul(
                        ps, lhsT=yt_sl(mi, ei),
                        rhs=at[:, ei, :],
                        start=(ei == 0), stop=(ei == EC - 1))
                nc.scalar.activation(
                    et[:, mi, :], ps,
                    mybir.ActivationFunctionType.Exp,
                    bias=zbias, scale=1.0 / float(np.sqrt(e)))
                if mi == 1:
                    nc.vector.tensor_add(esum, et[:, 0, :], et[:, 1, :])
                elif mi > 1:
                    nc.vector.tensor_add(esum, esum, et[:, mi, :])

        def o_phase(nb, et, esum16):
            # out[n, f] = (eT.T @ V) / (esum.T @ 1)
            for ns in range(NSUB):
                pss = psum_s.tile([P, 1], F32, tag="pss", name="pss")
                nc.tensor.matmul(
                    pss, lhsT=esum16[:, ns * P:(ns + 1) * P], rhs=ones16,
                    start=True, stop=True)
                pos = [psum_o.tile([P, 512], F32, tag="pso", name="pso")
                       for _ in range(FJ)]
                for mi in range(MC):
                    lhsT_e = et[:, mi, ns * P:(ns + 1) * P]
                    for fj in range(FJ):
                        nc.tensor.matmul(
                            pos[fj], lhsT=lhsT_e,
                            rhs=V_sb[:, mi, fj * 512:(fj + 1) * 512],
                            start=(mi == 0), stop=(mi == MC - 1))
                rec = small.tile([P, 1], F32)
                nc.vector.reciprocal(rec, pss)
                ob = out_pool.tile([P, f], F16)
                nc.vector.tensor_scalar_mul(ob[:, 0:512], pos[0], rec)
                nc.scalar.activation(
                    ob[:, 512:1024], pos[1],
                    mybir.ActivationFunctionType.Copy, scale=rec)
                nc.gpsimd.dma_start(out_v[nb * NSUB + ns], ob)

        for nb in range(NB):
            at = at_pool.tile([P, EC, nblk], F16, tag="at", name="at")
            if nb == 0:
                a0_phase(at)
                v_phase()
            else:
                a_phase(nb, at)
            et = et_pool.tile([P, MC, nblk], F16, tag="et", name="et")
            esum = es_pool.tile([P, nblk], F32, tag="es", name="es")
            s_phase(at, et, esum)
            esum16 = es_pool.tile([P, nblk], F16, tag="es16", name="es16")
            nc.vector.tensor_copy(esum16, esum)
            o_phase(nb, et, esum16)

    nc.compile()
    return nc


_CACHE = {}


def _swz(xT, inner):
    # [1024, cols] -> partition-major [128, ...] with `inner`-sized
    # contiguous column blocks per (chunk, partition) row
    d, cols = xT.shape
    nb = cols // inner
    v = xT.reshape(d // P, P, nb, inner).transpose(1, 2, 0, 3)
    return np.ascontiguousarray(v.reshape(P, -1))


def _fold_and_shard(q, y, Wq, Wk, Wv):
    q = np.asarray(q, dtype=np.float32)
    y = np.asarray(y, dtype=np.float32)
    Wq = np.asarray(Wq, dtype=np.float32)
    Wk = np.asarray(Wk, dtype=np.float32)
    Wv = np.asarray(Wv, dtype=np.float32)
    M = _swz((Wq @ Wk.T).astype(np.float16), E // 4)
    N = _swz((Wk @ Wv).astype(np.float16), F)
    in_maps = []
    for b in range(B):
        qTb = q[b].T.astype(np.float16)
        in_maps.append({
            "qT0": _swz(qTb[:, 0:NBLK], 256),
            "qTr": _swz(np.ascontiguousarray(qTb[:, NBLK:]), NBLK),
            "yT": _swz(y[b].T.astype(np.float16), NBLK),
            "M": M, "N": N,
        })
    return in_maps


def kernel(q, y, Wq, Wk, Wv):
    if "nc" not in _CACHE:
        _CACHE["nc"] = build_program()
    nc = _CACHE["nc"]
    in_maps = _fold_and_shard(q, y, Wq, Wk, Wv)
    res = run_bass_kernel_spmd(nc, in_maps, core_ids=list(range(B)))
    return np.stack(
        [res.results[b]["out"].astype(np.float32) for b in range(B)], axis=0)
